# revision 8
# baseline (speedup 1.0000x reference)
"""Mixtral MoE MLP (T=8192, H=2048, I=4096, E=8, top-2) on 8 TRN2 NeuronCores.

Strategy: tensor-parallel over intermediate_size (TP8). Every core handles
ALL routed token-expert pairs but only I/8 = 512 of the 4096 intermediate
features of each expert:
  gate/up[c] = ws[e][rows c*512:(c+1)*512 of gate and of up] @ x
  act[c]     = silu(gate[c]) * up[c]
  y_c        = w2s[e][:, c*512:(c+1)*512] @ act[c]      (partial over I)
and the host sums the 8 partial outputs. This gives EXACT load balance
(every core runs an identical 16384-pair schedule regardless of routing
imbalance), needs no token padding beyond 8-alignment per expert, and
shrinks the per-expert weight slice to 6.3MB so weights stream exactly once.

The router (tiny) runs on host in float64; token dispatch/combine is the
host-side shard/unshard step.

Device schedule per core: token blocks of <=512 (near-uniform within each
expert segment), software-pipelined one block deep: GEMM1(block i+1) is
emitted before GEMM2(block i) so the SiLU/mul drain of block i hides under
GEMM1 compute. All DRAM I/O is block-major so every x-load and y-store is
one large contiguous-per-partition DMA; y is staged per block in SBUF.
Weight prefetch rides the sync queue in ~1MB chunks spread across the
previous segment's blocks so no queue ever carries a burst that delays a
load the PE is about to need.
"""

import numpy as np

T, H, I, E = 8192, 2048, 4096, 8
TOP_K = 2
P = 128
KH = H // P            # 16 K-tiles for GEMM1 (contraction over H)
IC = I // E            # 512 intermediate features per core
NPAIR = IC // P        # 4  gate/up 128-row pair blocks per core
KI = IC // P           # 4  K-tiles for GEMM2 (contraction over IC)
NH = H // P            # 16 output row blocks of GEMM2
BLOCK = 512            # moving-operand / PSUM bank width

_module_cache = {}


def _blocks_for(cnt, narrow_first=False):
    """Near-uniform block widths <= BLOCK covering cnt tokens (cnt % 8 == 0),
    each width a multiple of 8. narrow_first carves a 256-wide lead block
    (same total block count) so the kernel's first matmul chain needs half
    the startup DMA bytes; only used for the first scheduled segment."""
    if cnt == 0:
        return []
    n = -(-cnt // BLOCK)
    lead = []
    if narrow_first and n > 1 and cnt - 256 <= (n - 1) * BLOCK:
        lead = [256]
        cnt -= 256
        n -= 1
    c8 = cnt // 8
    base, rem = divmod(c8, n)
    widths = lead + [(base + 1) * 8] * rem + [base * 8] * (n - rem)
    out = []
    off = 0
    for w in widths:
        out.append((off, w))
        off += w
    return out


def _sched_for(counts):
    """Flat block schedule: (seg_idx, tok_off, bw, xoff, yoff, first, last)
    plus seg list and totals."""
    segs = [e for e in range(E) if counts[e]]
    sched = []
    tok_off = 0
    xoff = 0
    yoff = 0
    for si, e in enumerate(segs):
        blocks = _blocks_for(counts[e], narrow_first=(si == 0))
        for bi, (boff, bw) in enumerate(blocks):
            sched.append((si, tok_off + boff, bw, xoff, yoff, bi == 0))
            xoff += KH * bw
            yoff += NH * bw
        tok_off += counts[e]
    return segs, sched, xoff, yoff


def _build_module(counts):
    """counts: per-expert padded token counts (each % 8 == 0)."""
    import concourse.mybir as mybir
    import concourse.tile as tile
    from concourse import bacc
    from contextlib import ExitStack

    fp16 = mybir.dt.float16
    fp32 = mybir.dt.float32

    segs, sched, TOTX, TOTY = _sched_for(counts)

    nc = bacc.Bacc("TRN2", target_bir_lowering=False, debug=False)

    xt = nc.dram_tensor("xt", [P, TOTX], fp16, kind="ExternalInput")
    w1 = nc.dram_tensor("w1", [E, P, NPAIR, KH, 2 * P], fp16,
                        kind="ExternalInput")
    w2 = nc.dram_tensor("w2", [E, P, NH, KI, P], fp16, kind="ExternalInput")
    yt = nc.dram_tensor("yt", [P, TOTY], fp16, kind="ExternalOutput")

    act_fn = mybir.ActivationFunctionType.Silu

    with tile.TileContext(nc) as tc, ExitStack() as ctx:
        xpool = ctx.enter_context(tc.tile_pool(name="xs", bufs=2))
        apool = ctx.enter_context(tc.tile_pool(name="act", bufs=2))
        w1pool = ctx.enter_context(tc.tile_pool(name="w1p", bufs=2))
        w2pool = ctx.enter_context(tc.tile_pool(name="w2p", bufs=2))
        tpool = ctx.enter_context(tc.tile_pool(name="tmp", bufs=2))
        ypool = ctx.enter_context(tc.tile_pool(name="yst", bufs=2))
        ps1 = ctx.enter_context(tc.tile_pool(name="ps1", bufs=2, space="PSUM"))
        ps2 = ctx.enter_context(tc.tile_pool(name="ps2", bufs=3, space="PSUM"))
        w1t = {}
        w2t = {}

        def alloc_weights(si):
            w1t[si] = w1pool.tile([P, NPAIR, KH, 2 * P], fp16, name="w1t")
            w2t[si] = w2pool.tile([P, NH, KI, P], fp16, name="w2t")

        def weight_chunks(si):
            """Weight DMAs for segs[si] as ~1MB thunks (sync queue)."""
            e = segs[si]
            out = []
            for p in range(NPAIR):
                out.append(lambda p=p: nc.sync.dma_start(
                    w1t[si][:, p, :, :], w1[e, :, p, :, :]))
            out.append(lambda: nc.sync.dma_start(w2t[si][:], w2[e]))
            return out

        def emit_g1(blk, pending):
            si, _, bw, xoff, _, _ = blk
            xs = xpool.tile([P, KH * bw], fp16)
            if xoff == 0:
                # startup: stream the first pair's operands in fine chunks
                # in consumption order, split across both queues, so the
                # first matmul starts as early as possible and the (still
                # cold-clocked) PE never outruns the stream
                e = segs[si]
                for ci, (k0, k1) in enumerate(
                        [(0, 1), (1, 2), (2, 4), (4, 6), (6, 8),
                         (8, 12), (12, 16)]):
                    wq = nc.sync if ci % 2 == 0 else nc.scalar
                    xq = nc.scalar if ci % 2 == 0 else nc.sync
                    wq.dma_start(w1t[si][:, 0, k0:k1, :],
                                 w1[e, :, 0, k0:k1, :])
                    xq.dma_start(xs[:, k0 * bw:k1 * bw],
                                 xt[:, k0 * bw:k1 * bw])
                for p in range(1, NPAIR):
                    q = nc.sync if p % 2 else nc.scalar
                    q.dma_start(w1t[si][:, p, :, :], w1[e, :, p, :, :])
                nc.scalar.dma_start(w2t[si][:], w2[e])
            else:
                nc.sync.dma_start(xs[:], xt[:, xoff:xoff + KH * bw])
            for th in pending:
                th()
            actT = apool.tile([P, NPAIR, bw], fp16)
            wt = w1t[si]
            for p in range(NPAIR):
                pg = ps1.tile([P, bw], fp32)
                pu = ps1.tile([P, bw], fp32)
                for k in range(KH):
                    nc.tensor.matmul(pg[:], wt[:, p, k, 0:P],
                                     xs[:, k * bw:(k + 1) * bw],
                                     start=(k == 0), stop=(k == KH - 1))
                for k in range(KH):
                    nc.tensor.matmul(pu[:], wt[:, p, k, P:2 * P],
                                     xs[:, k * bw:(k + 1) * bw],
                                     start=(k == 0), stop=(k == KH - 1))
                tmp = tpool.tile([P, bw], fp32)
                nc.scalar.activation(tmp[:], pg[:], act_fn)
                nc.vector.tensor_mul(actT[:, p, :], tmp[:], pu[:])
            return actT

        def emit_g2(blk, actT, last=False):
            si, _, bw, _, yoff, _ = blk
            wt = w2t[si]
            ys = ypool.tile([P, NH * bw], fp16)
            for h in range(NH):
                ps = ps2.tile([P, bw], fp32)
                for k2 in range(KI):
                    nc.tensor.matmul(ps[:], wt[:, h, k2, :], actT[:, k2, :],
                                     start=(k2 == 0), stop=(k2 == KI - 1))
                nc.vector.tensor_copy(ys[:, h * bw:(h + 1) * bw], ps[:])
            # store per block on the scalar queue so the sync queue (x +
            # weights) is never stuck behind a store that waits on the DVE
            # drain; the final block stores in 4 chunks so the last chunk
            # only waits on the last 4 casts
            if last:
                for q in range(0, NH, 2):
                    nc.scalar.dma_start(
                        yt[:, yoff + q * bw:yoff + (q + 2) * bw],
                        ys[:, q * bw:(q + 2) * bw])
            else:
                nc.scalar.dma_start(yt[:, yoff:yoff + NH * bw], ys[:])

        seg_nblk = {}
        for blk in sched:
            seg_nblk[blk[0]] = seg_nblk.get(blk[0], 0) + 1

        alloc_weights(0)
        seen = {}
        prev = None
        nxt_chunks = {}
        for blk in sched:
            si = blk[0]
            j = seen.get(si, 0)
            seen[si] = j + 1
            pending = []
            if si + 1 < len(segs):
                if j == 0:
                    alloc_weights(si + 1)
                    nxt_chunks[si] = weight_chunks(si + 1)
                nblk = seg_nblk[si]
                # during segment 0 the startup stream owns both queues:
                # defer the next segment's chunks past block 0
                j0 = 1 if (si == 0 and nblk > 1) else 0
                nspread = nblk - j0
                chunks = nxt_chunks[si]
                if j >= j0:
                    lo = (j - j0) * len(chunks) // nspread
                    hi = (j - j0 + 1) * len(chunks) // nspread
                    pending.extend(chunks[lo:hi])
            actT = emit_g1(blk, pending)
            if prev is not None:
                emit_g2(*prev)
            prev = (blk, actT)
        if prev is not None:
            emit_g2(*prev, last=True)

    nc.compile()
    return nc


def _route(hidden_states, router_w):
    """Replicate reference routing: softmax -> top-2 -> renormalize."""
    logits = hidden_states.astype(np.float64) @ router_w.astype(np.float64).T
    order = np.argsort(-logits, axis=1, kind="stable")
    top2 = order[:, :TOP_K]                                   # [T, 2]
    m = logits.max(axis=1, keepdims=True)
    p = np.exp(logits - m)
    p /= p.sum(axis=1, keepdims=True)
    w = np.take_along_axis(p, top2, axis=1)
    w = w / w.sum(axis=1, keepdims=True)                      # [T, 2]
    return top2, w


def _prep_w1(ws):
    # ws: [E, 2I, H] fp32 -> per-core [E, P(part=H%128), NPAIR, KH, 2P]
    w16 = ws.astype(np.float16)
    # I index = c*512 + p*128 + r ; H index = k*128 + h
    g = w16[:, :I].reshape(E, E, NPAIR, P, KH, P)    # e, c, pair, r, k, h
    u = w16[:, I:].reshape(E, E, NPAIR, P, KH, P)
    g = np.ascontiguousarray(g.transpose(1, 0, 5, 2, 4, 3))  # c,e,h,pair,k,r
    u = np.ascontiguousarray(u.transpose(1, 0, 5, 2, 4, 3))
    out = np.empty((E, E, P, NPAIR, KH, 2 * P), dtype=np.float16)
    out[:, :, :, :, :, :P] = g
    out[:, :, :, :, :, P:] = u
    return out


def _prep_w2(w2s):
    # w2s: [E, H, I] fp32 -> per-core [E, P(part=I-within-k2), NH, KI, P(H)]
    w16 = w2s.astype(np.float16)
    # H index = h*128 + m ; I index = c*512 + k2*128 + p
    w = w16.reshape(E, NH, P, E, KI, P)              # e, h, m, c, k2, p
    return np.ascontiguousarray(w.transpose(3, 0, 5, 1, 4, 2))  # c,e,p,h,k2,m


def _ensure_ntff_hook():
    """Register the axon NTFF profile hook if the image's antenv lacks it."""
    import sys, types
    try:
        from antenv.axon_hooks import get_axon_ntff_profile_hook  # noqa: F401
        return
    except ImportError:
        pass
    try:
        from trn_agent_boot.trn_boot import _ntff_profile_via_ctypes
        hook = _ntff_profile_via_ctypes("/opt/axon/libaxon_pjrt.so")
    except Exception:
        hook = None
    mod = types.ModuleType("antenv.axon_hooks")
    mod.get_axon_ntff_profile_hook = lambda: hook
    mod.set_axon_ntff_profile_hook = lambda h: None
    sys.modules["antenv.axon_hooks"] = mod


def _run(hidden_states, router_w, ws, w2s, trace=False):
    from concourse.bass_utils import run_bass_kernel_spmd

    # register unconditionally: the harness may enable tracing via the
    # BASS_TRACE env var rather than the trace kwarg
    _ensure_ntff_hook()

    hidden_states = np.asarray(hidden_states, dtype=np.float32)
    router_w = np.asarray(router_w, dtype=np.float32)
    ws = np.asarray(ws, dtype=np.float32)
    w2s = np.asarray(w2s, dtype=np.float32)

    top2, topw = _route(hidden_states, router_w)

    tok_idx = []        # per-expert token ids
    tok_w = []          # per-expert combine weights
    for e in range(E):
        rows, which = np.nonzero(top2 == e)
        tok_idx.append(rows)
        tok_w.append(topw[rows, which])

    counts = tuple(-(-len(ix) // 8) * 8 for ix in tok_idx)
    segs, sched, TOTX, TOTY = _sched_for(counts)

    if counts not in _module_cache:
        _module_cache[counts] = _build_module(counts)
    nc = _module_cache[counts]

    hidden16 = hidden_states.astype(np.float16)
    Ctot = sum(counts)
    x_pad = np.zeros((Ctot, H), dtype=np.float16)
    off = 0
    offsets = []
    for e in range(E):
        offsets.append(off)
        rows = tok_idx[e]
        x_pad[off:off + len(rows)] = hidden16[rows]
        off += counts[e]

    # block-major flat x: block at xoff holds [KH, bw] per partition
    xt = np.empty((P, TOTX), dtype=np.float16)
    for si, tok_off, bw, xoff, yoff, first in sched:
        xb = x_pad[tok_off:tok_off + bw].reshape(bw, KH, P)
        xt[:, xoff:xoff + KH * bw] = (
            xb.transpose(2, 1, 0).reshape(P, KH * bw))

    w1p = _prep_w1(ws)
    w2p = _prep_w2(w2s)
    in_maps = [{"xt": xt, "w1": w1p[c], "w2": w2p[c]} for c in range(E)]

    res = run_bass_kernel_spmd(nc, in_maps, core_ids=list(range(E)),
                               trace=trace)

    # sum partial outputs over cores (fp32), then unshard block-major y
    ysum = np.zeros((P, TOTY), dtype=np.float32)
    for c in range(E):
        ysum += res.results[c]["yt"]
    y = np.empty((Ctot, H), dtype=np.float32)
    for si, tok_off, bw, xoff, yoff, first in sched:
        yb = ysum[:, yoff:yoff + NH * bw].reshape(P, NH, bw)
        y[tok_off:tok_off + bw] = yb.transpose(2, 1, 0).reshape(bw, H)

    out = np.zeros(hidden_states.shape, dtype=np.float32)
    for e in range(E):
        rows = tok_idx[e]
        if not len(rows):
            continue
        o = offsets[e]
        out[rows] += tok_w[e][:, None].astype(np.float32) * y[o:o + len(rows)]
    return out, res


def kernel(hidden_states, router_w, ws, w2s):
    out, _ = _run(hidden_states, router_w, ws, w2s, trace=False)
    return out


# revision 9
# speedup vs baseline: 1.0461x; 1.0461x over previous
"""Mixtral MoE MLP (T=8192, H=2048, I=4096, E=8, top-2) on 8 TRN2 NeuronCores.

Strategy: tensor-parallel over intermediate_size (TP8). Every core handles
ALL routed token-expert pairs but only I/8 = 512 of the 4096 intermediate
features of each expert:
  gate/up[c] = ws[e][rows c*512:(c+1)*512 of gate and of up] @ x
  act[c]     = silu(gate[c]) * up[c]
  y_c        = w2s[e][:, c*512:(c+1)*512] @ act[c]      (partial over I)
and the host sums the 8 partial outputs. This gives EXACT load balance
(every core runs an identical 16384-pair schedule regardless of routing
imbalance), needs no token padding beyond 8-alignment per expert, and
shrinks the per-expert weight slice to 6.3MB so weights stream exactly once.

The router (tiny) runs on host in float64; token dispatch/combine is the
host-side shard/unshard step.

Device schedule per core: token blocks of <=512 (near-uniform within each
expert segment), software-pipelined one block deep: GEMM1(block i+1) is
emitted before GEMM2(block i) so the SiLU/mul drain of block i hides under
GEMM1 compute. All DRAM I/O is block-major so every x-load and y-store is
one large contiguous-per-partition DMA; y is staged per block in SBUF.
Weight prefetch rides the sync queue in ~1MB chunks spread across the
previous segment's blocks so no queue ever carries a burst that delays a
load the PE is about to need.
"""

import numpy as np

T, H, I, E = 8192, 2048, 4096, 8
TOP_K = 2
P = 128
KH = H // P            # 16 K-tiles for GEMM1 (contraction over H)
IC = I // E            # 512 intermediate features per core
NPAIR = IC // P        # 4  gate/up 128-row pair blocks per core
KI = IC // P           # 4  K-tiles for GEMM2 (contraction over IC)
NH = H // P            # 16 output row blocks of GEMM2
BLOCK = 512            # moving-operand / PSUM bank width

_module_cache = {}


def _blocks_for(cnt):
    """Near-uniform block widths <= BLOCK covering cnt tokens (cnt % 8 == 0),
    each width a multiple of 8."""
    if cnt == 0:
        return []
    n = -(-cnt // BLOCK)
    c8 = cnt // 8
    base, rem = divmod(c8, n)
    widths = [(base + 1) * 8] * rem + [base * 8] * (n - rem)
    out = []
    off = 0
    for w in widths:
        out.append((off, w))
        off += w
    return out


def _sched_for(counts):
    """Flat block schedule: (seg_idx, tok_off, bw, xoff, yoff, first, last)
    plus seg list and totals."""
    segs = [e for e in range(E) if counts[e]]
    sched = []
    tok_off = 0
    xoff = 0
    yoff = 0
    for si, e in enumerate(segs):
        blocks = _blocks_for(counts[e])
        for bi, (boff, bw) in enumerate(blocks):
            sched.append((si, tok_off + boff, bw, xoff, yoff, bi == 0))
            xoff += KH * bw
            yoff += NH * bw
        tok_off += counts[e]
    return segs, sched, xoff, yoff


def _build_module(counts):
    """counts: per-expert padded token counts (each % 8 == 0)."""
    import concourse.mybir as mybir
    import concourse.tile as tile
    from concourse import bacc
    from contextlib import ExitStack

    fp16 = mybir.dt.float16
    fp32 = mybir.dt.float32

    segs, sched, TOTX, TOTY = _sched_for(counts)

    nc = bacc.Bacc("TRN2", target_bir_lowering=False, debug=False)

    xt = nc.dram_tensor("xt", [P, TOTX], fp16, kind="ExternalInput")
    w1 = nc.dram_tensor("w1", [E, P, NPAIR, KH, 2 * P], fp16,
                        kind="ExternalInput")
    w2 = nc.dram_tensor("w2", [E, P, NH, KI, P], fp16, kind="ExternalInput")
    yt = nc.dram_tensor("yt", [P, TOTY], fp16, kind="ExternalOutput")

    act_fn = mybir.ActivationFunctionType.Silu

    with tile.TileContext(nc) as tc, ExitStack() as ctx:
        xpool = ctx.enter_context(tc.tile_pool(name="xs", bufs=2))
        apool = ctx.enter_context(tc.tile_pool(name="act", bufs=2))
        w1pool = ctx.enter_context(tc.tile_pool(name="w1p", bufs=2))
        w2pool = ctx.enter_context(tc.tile_pool(name="w2p", bufs=2))
        tpool = ctx.enter_context(tc.tile_pool(name="tmp", bufs=2))
        ypool = ctx.enter_context(tc.tile_pool(name="yst", bufs=2))
        ps1 = ctx.enter_context(tc.tile_pool(name="ps1", bufs=2, space="PSUM"))
        ps2 = ctx.enter_context(tc.tile_pool(name="ps2", bufs=3, space="PSUM"))
        w1t = {}
        w2t = {}

        def alloc_weights(si):
            w1t[si] = w1pool.tile([P, NPAIR, KH, 2 * P], fp16, name="w1t")
            w2t[si] = w2pool.tile([P, NH, KI, P], fp16, name="w2t")

        def weight_chunks(si):
            """Weight DMAs for segs[si] as ~1MB thunks (sync queue)."""
            e = segs[si]
            out = []
            for p in range(NPAIR):
                out.append(lambda p=p: nc.sync.dma_start(
                    w1t[si][:, p, :, :], w1[e, :, p, :, :]))
            out.append(lambda: nc.sync.dma_start(w2t[si][:], w2[e]))
            return out

        def emit_g1(blk, pending):
            si, _, bw, xoff, _, _ = blk
            xs = xpool.tile([P, KH * bw], fp16)
            if xoff == 0:
                # startup: stream the first pair's operands in fine chunks
                # in consumption order, split across both queues, so the
                # first matmul starts as early as possible and the (still
                # cold-clocked) PE never outruns the stream
                e = segs[si]
                for ci, (k0, k1) in enumerate(
                        [(0, 1), (1, 2), (2, 4), (4, 6), (6, 8),
                         (8, 12), (12, 16)]):
                    wq = nc.sync if ci % 2 == 0 else nc.scalar
                    xq = nc.scalar if ci % 2 == 0 else nc.sync
                    wq.dma_start(w1t[si][:, 0, k0:k1, :],
                                 w1[e, :, 0, k0:k1, :])
                    xq.dma_start(xs[:, k0 * bw:k1 * bw],
                                 xt[:, k0 * bw:k1 * bw])
                for p in range(1, NPAIR):
                    q = nc.sync if p % 2 else nc.scalar
                    q.dma_start(w1t[si][:, p, :, :], w1[e, :, p, :, :])
                nc.scalar.dma_start(w2t[si][:], w2[e])
            else:
                nc.sync.dma_start(xs[:], xt[:, xoff:xoff + KH * bw])
            for th in pending:
                th()
            actT = apool.tile([P, NPAIR, bw], fp16)
            wt = w1t[si]
            for p in range(NPAIR):
                pg = ps1.tile([P, bw], fp32)
                pu = ps1.tile([P, bw], fp32)
                for k in range(KH):
                    nc.tensor.matmul(pg[:], wt[:, p, k, 0:P],
                                     xs[:, k * bw:(k + 1) * bw],
                                     start=(k == 0), stop=(k == KH - 1))
                for k in range(KH):
                    nc.tensor.matmul(pu[:], wt[:, p, k, P:2 * P],
                                     xs[:, k * bw:(k + 1) * bw],
                                     start=(k == 0), stop=(k == KH - 1))
                tmp = tpool.tile([P, bw], fp32)
                nc.scalar.activation(tmp[:], pg[:], act_fn)
                nc.vector.tensor_mul(actT[:, p, :], tmp[:], pu[:])
            return actT

        def emit_g2(blk, actT, last=False):
            si, _, bw, _, yoff, _ = blk
            wt = w2t[si]
            ys = ypool.tile([P, NH * bw], fp16)
            for h in range(NH):
                ps = ps2.tile([P, bw], fp32)
                for k2 in range(KI):
                    nc.tensor.matmul(ps[:], wt[:, h, k2, :], actT[:, k2, :],
                                     start=(k2 == 0), stop=(k2 == KI - 1))
                nc.vector.tensor_copy(ys[:, h * bw:(h + 1) * bw], ps[:])
            # store per block on the scalar queue so the sync queue (x +
            # weights) is never stuck behind a store that waits on the DVE
            # drain; the final block stores in 4 chunks so the last chunk
            # only waits on the last 4 casts
            if last:
                for q in range(0, NH, 2):
                    nc.scalar.dma_start(
                        yt[:, yoff + q * bw:yoff + (q + 2) * bw],
                        ys[:, q * bw:(q + 2) * bw])
            else:
                nc.scalar.dma_start(yt[:, yoff:yoff + NH * bw], ys[:])

        seg_nblk = {}
        for blk in sched:
            seg_nblk[blk[0]] = seg_nblk.get(blk[0], 0) + 1

        alloc_weights(0)
        seen = {}
        prev = None
        nxt_chunks = {}
        for blk in sched:
            si = blk[0]
            j = seen.get(si, 0)
            seen[si] = j + 1
            pending = []
            if si + 1 < len(segs):
                if j == 0:
                    alloc_weights(si + 1)
                    nxt_chunks[si] = weight_chunks(si + 1)
                nblk = seg_nblk[si]
                # during segment 0 the startup stream owns both queues:
                # defer the next segment's chunks past block 0
                j0 = 1 if (si == 0 and nblk > 1) else 0
                nspread = nblk - j0
                chunks = nxt_chunks[si]
                if j >= j0:
                    lo = (j - j0) * len(chunks) // nspread
                    hi = (j - j0 + 1) * len(chunks) // nspread
                    pending.extend(chunks[lo:hi])
            actT = emit_g1(blk, pending)
            if prev is not None:
                emit_g2(*prev)
            prev = (blk, actT)
        if prev is not None:
            emit_g2(*prev, last=True)

    nc.compile()
    return nc


def _route(hidden_states, router_w):
    """Replicate reference routing: softmax -> top-2 -> renormalize."""
    logits = hidden_states.astype(np.float64) @ router_w.astype(np.float64).T
    order = np.argsort(-logits, axis=1, kind="stable")
    top2 = order[:, :TOP_K]                                   # [T, 2]
    m = logits.max(axis=1, keepdims=True)
    p = np.exp(logits - m)
    p /= p.sum(axis=1, keepdims=True)
    w = np.take_along_axis(p, top2, axis=1)
    w = w / w.sum(axis=1, keepdims=True)                      # [T, 2]
    return top2, w


def _prep_w1(ws):
    # ws: [E, 2I, H] fp32 -> per-core [E, P(part=H%128), NPAIR, KH, 2P]
    w16 = ws.astype(np.float16)
    # I index = c*512 + p*128 + r ; H index = k*128 + h
    g = w16[:, :I].reshape(E, E, NPAIR, P, KH, P)    # e, c, pair, r, k, h
    u = w16[:, I:].reshape(E, E, NPAIR, P, KH, P)
    g = np.ascontiguousarray(g.transpose(1, 0, 5, 2, 4, 3))  # c,e,h,pair,k,r
    u = np.ascontiguousarray(u.transpose(1, 0, 5, 2, 4, 3))
    out = np.empty((E, E, P, NPAIR, KH, 2 * P), dtype=np.float16)
    out[:, :, :, :, :, :P] = g
    out[:, :, :, :, :, P:] = u
    return out


def _prep_w2(w2s):
    # w2s: [E, H, I] fp32 -> per-core [E, P(part=I-within-k2), NH, KI, P(H)]
    w16 = w2s.astype(np.float16)
    # H index = h*128 + m ; I index = c*512 + k2*128 + p
    w = w16.reshape(E, NH, P, E, KI, P)              # e, h, m, c, k2, p
    return np.ascontiguousarray(w.transpose(3, 0, 5, 1, 4, 2))  # c,e,p,h,k2,m


def _ensure_ntff_hook():
    """Register the axon NTFF profile hook if the image's antenv lacks it."""
    import sys, types
    try:
        from antenv.axon_hooks import get_axon_ntff_profile_hook  # noqa: F401
        return
    except ImportError:
        pass
    try:
        from trn_agent_boot.trn_boot import _ntff_profile_via_ctypes
        hook = _ntff_profile_via_ctypes("/opt/axon/libaxon_pjrt.so")
    except Exception:
        hook = None
    mod = types.ModuleType("antenv.axon_hooks")
    mod.get_axon_ntff_profile_hook = lambda: hook
    mod.set_axon_ntff_profile_hook = lambda h: None
    sys.modules["antenv.axon_hooks"] = mod


def _run(hidden_states, router_w, ws, w2s, trace=False):
    from concourse.bass_utils import run_bass_kernel_spmd

    # register unconditionally: the harness may enable tracing via the
    # BASS_TRACE env var rather than the trace kwarg
    _ensure_ntff_hook()

    hidden_states = np.asarray(hidden_states, dtype=np.float32)
    router_w = np.asarray(router_w, dtype=np.float32)
    ws = np.asarray(ws, dtype=np.float32)
    w2s = np.asarray(w2s, dtype=np.float32)

    top2, topw = _route(hidden_states, router_w)

    tok_idx = []        # per-expert token ids
    tok_w = []          # per-expert combine weights
    for e in range(E):
        rows, which = np.nonzero(top2 == e)
        tok_idx.append(rows)
        tok_w.append(topw[rows, which])

    counts = tuple(-(-len(ix) // 8) * 8 for ix in tok_idx)
    segs, sched, TOTX, TOTY = _sched_for(counts)

    if counts not in _module_cache:
        _module_cache[counts] = _build_module(counts)
    nc = _module_cache[counts]

    hidden16 = hidden_states.astype(np.float16)
    Ctot = sum(counts)
    x_pad = np.zeros((Ctot, H), dtype=np.float16)
    off = 0
    offsets = []
    for e in range(E):
        offsets.append(off)
        rows = tok_idx[e]
        x_pad[off:off + len(rows)] = hidden16[rows]
        off += counts[e]

    # block-major flat x: block at xoff holds [KH, bw] per partition
    xt = np.empty((P, TOTX), dtype=np.float16)
    for si, tok_off, bw, xoff, yoff, first in sched:
        xb = x_pad[tok_off:tok_off + bw].reshape(bw, KH, P)
        xt[:, xoff:xoff + KH * bw] = (
            xb.transpose(2, 1, 0).reshape(P, KH * bw))

    w1p = _prep_w1(ws)
    w2p = _prep_w2(w2s)
    in_maps = [{"xt": xt, "w1": w1p[c], "w2": w2p[c]} for c in range(E)]

    res = run_bass_kernel_spmd(nc, in_maps, core_ids=list(range(E)),
                               trace=trace)

    # sum partial outputs over cores (fp32), then unshard block-major y
    ysum = np.zeros((P, TOTY), dtype=np.float32)
    for c in range(E):
        ysum += res.results[c]["yt"]
    y = np.empty((Ctot, H), dtype=np.float32)
    for si, tok_off, bw, xoff, yoff, first in sched:
        yb = ysum[:, yoff:yoff + NH * bw].reshape(P, NH, bw)
        y[tok_off:tok_off + bw] = yb.transpose(2, 1, 0).reshape(bw, H)

    out = np.zeros(hidden_states.shape, dtype=np.float32)
    for e in range(E):
        rows = tok_idx[e]
        if not len(rows):
            continue
        o = offsets[e]
        out[rows] += tok_w[e][:, None].astype(np.float32) * y[o:o + len(rows)]
    return out, res


def kernel(hidden_states, router_w, ws, w2s):
    out, _ = _run(hidden_states, router_w, ws, w2s, trace=False)
    return out
